# revision 6
# baseline (speedup 1.0000x reference)
"""Cross-attention kernel for 8 Trainium2 NeuronCores (Bass/Tile, SPMD).

Reference computation (per batch b of 4):
    K_proj = K[b] @ Wk.T + bk            # [2048, 1024]
    V_proj = V[b] @ Wv.T + bv            # [2048, 1024]
    S      = Q[b] @ K_proj.T / 32        # [1024, 2048]
    P      = softmax(S, axis=-1)
    ctx    = P @ V_proj                  # [1024, 1024]
    out[b] = ctx @ Wo.T + bo             # [1024, 1024]

Sharding: 8 cores = 4 batches x 2 core-pairs.  Within a pair, queries
are split in half (as inputs) AND the K/V projections are split by
keys: each core projects only its 1024 keys (its KT/VT inputs are the
pre-sliced halves), then the pair exchanges projected halves with two
AllGather collectives (bf16 payload, DRAM bounce buffers).  This
removes the projection-duplication of the pure data-parallel split:
per-core matmul work drops from 8.05 to 5.37 GMAC.

Key-order note: gathered key order is [even core's keys, odd core's
keys] == global order, identical on both cores, so softmax/ctx/out are
exactly the reference computation.

dtypes: projections computed in f32r (PSUM f32), written bf16 for the
exchange; S/ctx matmuls and transposes run bf16 (same PE rate as f32r,
transposes 1.5x cheaper); out-projection stays f32r.

This container's walrus accepts at most ONE sync-wait command per
instruction; PatchedTileContext + split_multi_waits legalize that.
"""

import numpy as np
import ml_dtypes

import concourse.bass as bass
import concourse.mybir as mybir
import concourse.tile as tile
from concourse.bass_utils import run_bass_kernel_spmd
from concourse.masks import make_identity
from bass_rust import ScopedClock, VectorClock
from contextlib import ExitStack

F32 = mybir.dt.float32
F32R = mybir.dt.float32r
BF16 = mybir.dt.bfloat16
AX = mybir.AxisListType.X
EXP = mybir.ActivationFunctionType.Exp

B = 4
D1 = 1024
D2 = 1280
LK = 2048
LKL = 1024        # keys per core (pair-local half)
LQ = 512          # queries per core
N_CORES = 8
SCALE = 1.0 / 32.0  # 1/sqrt(D1)
GROUPS = [[0, 1], [2, 3], [4, 5], [6, 7]]

NT1 = D1 // 128   # 8  d1 tiles
NT2 = D2 // 128   # 10 d2 tiles
NTK = LK // 128   # 16 key tiles (global)
NQ = LQ // 128    # 4  query tiles per core
NKBL = LKL // 512  # 2 local key blocks of 512


class PatchedTileContext(tile.TileContext):
    """Split the end-of-context drain into single-wait drains."""

    def _drain_and_barrier(self, tick_clock, wait_clock):
        gc = tick_clock.global_clock
        n = len(gc)
        for i in range(n):
            t = gc[i]
            if t > 0:
                vec = [0] * n
                vec[i] = t
                d = self.nc.sync.drain()
                wait_clock.add_sem_waits(
                    d.ins, ScopedClock({None: VectorClock(vec)})
                )
        self.nc.all_engine_barrier()
        assert self.sems is not None
        popped = self.nc._tile_sem_poison_stack.pop()
        assert popped is self._sem_poison
        self.nc.clear_and_free_semaphores(list(self.sems.allocated().values()))
        self.nc.all_engine_barrier()


def split_multi_waits(nc, limit=1):
    """Hoist waits beyond `limit` onto same-engine NoOps placed directly
    before the offending instruction."""
    n_split = 0
    for fn in nc.m.functions:
        for blk in fn.blocks:
            il = blk.instructions
            newlist = []
            changed = False
            for inst in il:
                si = inst.sync_info
                ow = list(si.on_wait) if si is not None else []
                if len(ow) > limit:
                    for k, w in enumerate(ow[:-limit]):
                        nop = mybir.InstNoOp(
                            name=f"{inst.name}-ws{k}", ins=[], outs=[]
                        )
                        nop.engine = inst.engine
                        nop.sync_info = mybir.SyncInfo(on_wait=[w], on_update=[])
                        newlist.append(nop)
                        n_split += 1
                    inst.sync_info = mybir.SyncInfo(
                        on_wait=ow[-limit:], on_update=list(si.on_update)
                    )
                    changed = True
                newlist.append(inst)
            if changed:
                del il[:]
                il.extend(newlist)
    return n_split


def build_program(n_rounds=1):
    nc = bass.Bass(num_devices=N_CORES)

    QT = nc.dram_tensor("QT", [D1, LQ], BF16, kind="ExternalInput")
    KT = nc.dram_tensor("KT", [D2, LKL], F32, kind="ExternalInput")
    VT = nc.dram_tensor("VT", [D2, LKL], F32, kind="ExternalInput")
    WkT = nc.dram_tensor("WkT", [D2, D1], F32, kind="ExternalInput")
    WvT = nc.dram_tensor("WvT", [D2, D1], F32, kind="ExternalInput")
    WoT = nc.dram_tensor("WoT", [D1, D1], F32, kind="ExternalInput")
    bkbo = nc.dram_tensor("bkbo", [128, 2 * NT1], F32, kind="ExternalInput")
    bvB = nc.dram_tensor("bvB", [128, D1], F32, kind="ExternalInput")
    outT = nc.dram_tensor("outT", [D1, LQ], F32, kind="ExternalOutput")

    with PatchedTileContext(nc) as tc:
        es_stats = ExitStack()
        stats = es_stats.enter_context(tc.tile_pool(name="stats", bufs=1))
        statv = es_stats.enter_context(tc.tile_pool(name="statv", bufs=8))
        ident = stats.tile([128, 128], F32)
        make_identity(nc, ident[:])
        identb = stats.tile([128, 128], BF16)
        nc.vector.tensor_copy(identb[:], ident[:])
        bias_t = stats.tile([128, 2 * NT1], F32)
        nc.sync.dma_start(bias_t[:], bkbo[:])
        bvB_t = stats.tile([128, D1], F32)

        def emit_round(rnd):
            sfx = f"_{rnd}"
            es_pwv = ExitStack()     # wvT (prefetched during A)
            es_a = ExitStack()       # wkT + KT stream
            es_kl = ExitStack()      # kp_loc staging
            es_p1 = ExitStack()      # kpT + qT (right side)
            es_sm = ExitStack()      # esb
            es_pt = ExitStack()      # pT (right side)
            es_c = ExitStack()       # VT stream
            es_vl = ExitStack()      # vp_loc staging
            es_vp = ExitStack()      # vp (right side)
            es_tail = ExitStack()    # woT + ctxT + osb
            es_ppa = ExitStack()
            es_pps = ExitStack()
            es_ppt = ExitStack()
            es_ppcd = ExitStack()
            es_dram = ExitStack()

            dpool = es_dram.enter_context(
                tc.tile_pool(name="dram" + sfx, bufs=1, space="DRAM")
            )
            xbk = dpool.tile([128, NT1, LKL], BF16, tag="xbk", name="xbk" + sfx)
            gbk = dpool.tile([256, NT1, LKL], BF16, tag="gbk", name="gbk" + sfx)
            xbv = dpool.tile([128, NT1, D1], BF16, tag="xbv", name="xbv" + sfx)
            gbv = dpool.tile([256, NT1, D1], BF16, tag="gbv", name="gbv" + sfx)

            # ---- phase A: kp_loc = Wk @ K_loc.T  [d1, LKL] (bf16) --------
            vpp = es_vp.enter_context(
                tc.tile_pool(name="vpp" + sfx, bufs=1, side="right")
            )
            vp = vpp.tile([128, NTK, D1], BF16)
            p1 = es_p1.enter_context(
                tc.tile_pool(name="p1" + sfx, bufs=1, side="right")
            )
            kpT = p1.tile([128, NT1, LK], BF16)
            qT = p1.tile([128, NT1, LQ], BF16)
            vploc = es_vl.enter_context(tc.tile_pool(name="vpl" + sfx, bufs=1))
            vp_loc = vploc.tile([128, NT1, D1], BF16)
            kploc = es_kl.enter_context(tc.tile_pool(name="kpl" + sfx, bufs=1))
            kp_loc = kploc.tile([128, NT1, LKL], BF16)

            pa = es_a.enter_context(tc.tile_pool(name="pa" + sfx, bufs=1))
            pa_s = es_a.enter_context(tc.tile_pool(name="pa_s" + sfx, bufs=2))
            ppa = es_ppa.enter_context(
                tc.tile_pool(name="ppa" + sfx, bufs=4, space="PSUM")
            )

            wk_t = [pa.tile([128, D1], F32, tag=f"wk{f}", name=f"wk{f}" + sfx)
                    for f in range(NT2)]
            wv_t = []
            for n in range(NKBL):
                ks = [pa_s.tile([128, 512], F32, tag=f"ks{f}", name=f"ks{f}_{n}" + sfx)
                      for f in range(NT2)]
                for f in range(NT2):
                    if n == 0:
                        nc.sync.dma_start(
                            wk_t[f][:].bitcast(F32R),
                            WkT[f * 128 : (f + 1) * 128, :].bitcast(F32R),
                        )
                    nc.sync.dma_start(
                        ks[f][:].bitcast(F32R),
                        KT[f * 128 : (f + 1) * 128, n * 512 : (n + 1) * 512].bitcast(F32R),
                    )
                for m in range(NT1):
                    ps = ppa.tile([128, 512], F32, tag="ppa")
                    for f in range(NT2):
                        nc.tensor.matmul(
                            ps[:],
                            wk_t[f][:, m * 128 : (m + 1) * 128].bitcast(F32R),
                            ks[f][:].bitcast(F32R),
                            start=(f == 0),
                            stop=(f == NT2 - 1),
                        )
                    nc.vector.tensor_scalar_add(
                        kp_loc[:, m, n * 512 : (n + 1) * 512],
                        ps[:],
                        bias_t[:, m : m + 1],
                    )
                if n == NKBL - 1:
                    # demoted loads: needed only from phase B / C onward
                    for f in range(NT1):
                        nc.sync.dma_start(
                            qT[:, f, :], QT[f * 128 : (f + 1) * 128, :]
                        )
                    nc.sync.dma_start(bvB_t[:], bvB[:])
            es_a.close()

            # ---- kpT exchange: bounce out, pair AllGather, read back ----
            nc.gpsimd.dma_start(xbk[:], kp_loc[:])
            nc.gpsimd.collective_compute(
                "AllGather",
                mybir.AluOpType.bypass,
                replica_groups=GROUPS,
                ins=[xbk[:]],
                outs=[gbk[:]],
            )
            nc.gpsimd.dma_start(kpT[:, :, 0:LKL], gbk[0:128])
            nc.gpsimd.dma_start(kpT[:, :, LKL:LK], gbk[128:256])
            es_kl.close()

            # ---- phase C: vp_loc = V_loc_proj  [LKL, d1] (bf16) ---------
            # emitted before S so the PE computes it while the kpT
            # collective is in flight.
            pwv = es_pwv.enter_context(tc.tile_pool(name="pwv" + sfx, bufs=1))
            for f in range(NT2):
                w = pwv.tile([128, D1], F32, tag=f"wv{f}", name=f"wv{f}" + sfx)
                nc.sync.dma_start(
                    w[:].bitcast(F32R),
                    WvT[f * 128 : (f + 1) * 128, :].bitcast(F32R),
                )
                wv_t.append(w)
            pc_s = es_c.enter_context(tc.tile_pool(name="pc_s" + sfx, bufs=2))
            for n in range(NKBL):
                vs = [pc_s.tile([128, 512], F32, tag=f"vs{f}", name=f"vs{f}_{n}" + sfx)
                      for f in range(NT2)]
                for f in range(NT2):
                    nc.sync.dma_start(
                        vs[f][:].bitcast(F32R),
                        VT[f * 128 : (f + 1) * 128, n * 512 : (n + 1) * 512].bitcast(F32R),
                    )
                for j in range(4):
                    kt = n * 4 + j
                    for dh in range(2):
                        ps = ppa.tile([128, 512], F32, tag="ppa")
                        for f in range(NT2):
                            nc.tensor.matmul(
                                ps[:],
                                vs[f][:, j * 128 : (j + 1) * 128].bitcast(F32R),
                                wv_t[f][:, dh * 512 : (dh + 1) * 512].bitcast(F32R),
                                start=(f == 0),
                                stop=(f == NT2 - 1),
                            )
                        nc.vector.tensor_add(
                            vp_loc[:, kt, dh * 512 : (dh + 1) * 512],
                            ps[:],
                            bvB_t[:, dh * 512 : (dh + 1) * 512],
                        )
            es_c.close()
            es_pwv.close()

            # ---- vp exchange --------------------------------------------
            nc.gpsimd.dma_start(xbv[:], vp_loc[:])
            nc.gpsimd.collective_compute(
                "AllGather",
                mybir.AluOpType.bypass,
                replica_groups=GROUPS,
                ins=[xbv[:]],
                outs=[gbv[:]],
            )
            nc.gpsimd.dma_start(vp[:, 0 : NT1, :], gbv[0:128])
            nc.gpsimd.dma_start(vp[:, NT1 : NTK, :], gbv[128:256])
            es_vl.close()

            # ---- phase B: S = qT.T @ kpT, softmax along k (bf16) --------
            sm = es_sm.enter_context(tc.tile_pool(name="sm" + sfx, bufs=1))
            esb = sm.tile([128, NQ, LK], BF16)
            es_ppa.close()
            pps = es_pps.enter_context(
                tc.tile_pool(name="pps" + sfx, bufs=3, space="PSUM")
            )
            ppt = es_ppt.enter_context(
                tc.tile_pool(name="ppt" + sfx, bufs=2, space="PSUM")
            )
            for m in range(NQ):
                ph = [pps.tile([128, 1024], F32, tag="pps", name=f"ps{m}h{h}" + sfx)
                      for h in range(2)]
                for n in range(LK // 512):
                    ps = ph[n // 2]
                    off = (n % 2) * 512
                    for f in range(NT1):
                        nc.tensor.matmul(
                            ps[:, off : off + 512],
                            qT[:, f, m * 128 : (m + 1) * 128],
                            kpT[:, f, n * 512 : (n + 1) * 512],
                            start=(f == 0),
                            stop=(f == NT1 - 1),
                        )
                mr = [statv.tile([128, 1], F32, tag=f"mr{h}", name=f"mr{m}h{h}" + sfx)
                      for h in range(2)]
                for h in range(2):
                    nc.vector.reduce_max(mr[h][:], ph[h][:], axis=AX)
                mraw = statv.tile([128, 1], F32, tag="mraw")
                nc.vector.tensor_max(mraw[:], mr[0][:], mr[1][:])
                mneg = statv.tile([128, 1], F32, tag="mneg")
                nc.scalar.mul(mneg[:], mraw[:], -SCALE)
                ls = [statv.tile([128, 1], F32, tag=f"ls{h}", name=f"ls{m}h{h}" + sfx)
                      for h in range(2)]
                for h in range(2):
                    nc.scalar.activation(
                        esb[:, m, h * 1024 : (h + 1) * 1024],
                        ph[h][:],
                        EXP,
                        bias=mneg[:],
                        scale=SCALE,
                        accum_out=ls[h][:],
                    )
                lsum = statv.tile([128, 1], F32, tag="lsum")
                nc.vector.tensor_add(lsum[:], ls[0][:], ls[1][:])
                rinv = statv.tile([128, 1], F32, tag="rinv")
                nc.vector.reciprocal(rinv[:], lsum[:])
                nc.vector.tensor_scalar_mul(
                    esb[:, m, :],
                    esb[:, m, :],
                    rinv[:],
                )
            es_p1.close()
            pt = es_pt.enter_context(
                tc.tile_pool(name="pt" + sfx, bufs=1, side="right")
            )
            pT = pt.tile([128, NTK, LQ], BF16)
            for m in range(NQ):
                for kt in range(NTK):
                    tp = ppt.tile([128, 128], BF16, tag="ppt")
                    nc.tensor.transpose(
                        tp[:], esb[:, m, kt * 128 : (kt + 1) * 128], identb[:]
                    )
                    nc.vector.tensor_copy(
                        pT[:, kt, m * 128 : (m + 1) * 128],
                        tp[:],
                    )
            es_sm.close()

            # ---- phase D: ctxT = V_proj.T @ P.T  [d, q] -----------------
            es_ppt.close()
            es_pps.close()
            ppc = es_ppcd.enter_context(
                tc.tile_pool(name="ppc" + sfx, bufs=4, space="PSUM")
            )
            ppd = es_ppcd.enter_context(
                tc.tile_pool(name="ppd" + sfx, bufs=4, space="PSUM")
            )
            ptail = es_tail.enter_context(tc.tile_pool(name="ptail" + sfx, bufs=1))
            posb = es_tail.enter_context(tc.tile_pool(name="posb" + sfx, bufs=2))
            ctxT = ptail.tile([128, NT1, LQ], F32)
            woT = ptail.tile([128, NT1, D1], F32)
            for f in range(NT1):
                nc.sync.dma_start(
                    woT[:, f, :].bitcast(F32R),
                    WoT[f * 128 : (f + 1) * 128, :].bitcast(F32R),
                )
            for dt in range(NT1):
                ps = ppd.tile([128, LQ], F32, tag="ppd")
                for kt in range(NTK):
                    nc.tensor.matmul(
                        ps[:],
                        vp[:, kt, dt * 128 : (dt + 1) * 128],
                        pT[:, kt, :],
                        start=(kt == 0),
                        stop=(kt == NTK - 1),
                    )
                nc.vector.tensor_copy(
                    ctxT[:, dt, :].bitcast(F32R), ps[:]
                )
            es_pt.close()
            es_vp.close()

            # ---- phase E: outT = Wo @ ctxT + bo  [e, q] -----------------
            for et in range(NT1):
                ps = ppc.tile([128, LQ], F32, tag="ppc")
                for dt in range(NT1):
                    nc.tensor.matmul(
                        ps[:],
                        woT[:, dt, et * 128 : (et + 1) * 128].bitcast(F32R),
                        ctxT[:, dt, :].bitcast(F32R),
                        start=(dt == 0),
                        stop=(dt == NT1 - 1),
                    )
                ob = posb.tile([128, LQ], F32, tag="osb")
                nc.vector.tensor_scalar_add(
                    ob[:], ps[:], bias_t[:, NT1 + et : NT1 + et + 1]
                )
                nc.sync.dma_start(outT[et * 128 : (et + 1) * 128, :], ob[:])
            es_ppcd.close()
            es_tail.close()
            es_dram.close()

        for rnd in range(n_rounds):
            emit_round(rnd)
        es_stats.close()

    split_multi_waits(nc)
    return nc


_PROGRAM = None


def _get_program():
    global _PROGRAM
    if _PROGRAM is None:
        _PROGRAM = build_program()
    return _PROGRAM


def build_in_maps(inputs):
    Q = np.asarray(inputs["Q"], dtype=np.float32)
    K = np.asarray(inputs["K"], dtype=np.float32)
    V = np.asarray(inputs["V"], dtype=np.float32)
    Wk = np.asarray(inputs["Wk"], dtype=np.float32)
    Wv = np.asarray(inputs["Wv"], dtype=np.float32)
    Wo = np.asarray(inputs["Wo"], dtype=np.float32)
    bk = np.asarray(inputs["bk"], dtype=np.float32)
    bv = np.asarray(inputs["bv"], dtype=np.float32)
    bo = np.asarray(inputs["bo"], dtype=np.float32)

    WkT_h = np.ascontiguousarray(Wk.T)            # [D2, D1]
    WvT_h = np.ascontiguousarray(Wv.T)
    WoT_h = np.ascontiguousarray(Wo.T)            # [D1, D1]
    bkbo_h = np.concatenate(
        [bk.reshape(NT1, 128).T, bo.reshape(NT1, 128).T], axis=1
    ).astype(np.float32).copy()
    bvB_h = np.ascontiguousarray(np.broadcast_to(bv, (128, D1)))
    KT_h = [np.ascontiguousarray(K[b].T) for b in range(B)]   # [D2, LK]
    VT_h = [np.ascontiguousarray(V[b].T) for b in range(B)]

    in_maps = []
    for c in range(N_CORES):
        b, h = divmod(c, 2)
        in_maps.append(
            {
                "QT": np.ascontiguousarray(
                    Q[b, h * LQ : (h + 1) * LQ, :].T
                ).astype(ml_dtypes.bfloat16),
                "KT": np.ascontiguousarray(
                    KT_h[b][:, h * LKL : (h + 1) * LKL]
                ),
                "VT": np.ascontiguousarray(
                    VT_h[b][:, h * LKL : (h + 1) * LKL]
                ),
                "WkT": WkT_h,
                "WvT": WvT_h,
                "WoT": WoT_h,
                "bkbo": bkbo_h,
                "bvB": bvB_h,
            }
        )
    return in_maps


def assemble_output(results):
    out = np.empty((B, 2 * LQ, D1), dtype=np.float32)
    for c in range(N_CORES):
        b, h = divmod(c, 2)
        out[b, h * LQ : (h + 1) * LQ, :] = results[c]["outT"].T
    return out


def kernel(Q, K, V, Wk, bk, Wv, bv, Wo, bo):
    inputs = dict(Q=Q, K=K, V=V, Wk=Wk, bk=bk, Wv=Wv, bv=bv, Wo=Wo, bo=bo)
    nc = _get_program()
    in_maps = build_in_maps(inputs)
    res = run_bass_kernel_spmd(nc, in_maps, list(range(N_CORES)))
    return assemble_output(res.results)
